# revision 53
# baseline (speedup 1.0000x reference)
"""DoRA Linear on 8 Trainium2 NeuronCores (Bass/Tile).

Reference computation (all fp32):
    new_v   = base_weight + SCALE * dora_B @ dora_A          [OUT, IN]
    scale_o = weight_m / ||new_v||_row                        [OUT]
    out     = x @ (scale_o[:, None] * new_v)^T + base_bias    [B, S, OUT]

Sharding: column-parallel over OUT across 8 cores (OUT/8 = 512 each).
base_weight, dora_B, weight_m, base_bias sharded; x, dora_A replicated.

Per-core device program (heavy math in float32r on the PE):
  1. Build W'^T = (W + SCALE*B@A)^T chunk-by-chunk in SBUF:
     PE matmul A^T@(2B^T) -> PSUM, DVE adds the W^T chunk and writes the
     f32r weight tile `wr` the main matmul consumes.
  2. Row norms: sq = wr*wr (DVE), then PE matmuls with sq as lhsT and an
     all-ones rhs (N=256) accumulate sum_i sq[i, o] over k-chunks,
     giving norms directly in o-partition column layout.  NOTE: f32/f32r
     matmuls whose PSUM output has fewer than 128 partitions compile but
     produce a NEFF the runtime refuses to load - keep M = 128.
  3. scale_col = weight_m / sqrt(norm2)  (ACT sqrt + DVE reciprocal/mul)
  4. Main matmul, output-transposed orientation:
     outT[o, m] = sum_k W'^T[k, o] * xT[k, m], PSUM-accumulated over 32
     k-chunks; eviction fuses *scale_o + bias_o in one DVE tensor_scalar
     (per-partition scalars).
Host: layout transposes in numpy (no FLOPs), final gather + transpose.
"""

import numpy as np

import concourse.mybir as mybir
import concourse.tile as tile
from concourse import bacc
from concourse.bass_utils import run_bass_kernel_spmd
from concourse.masks import make_identity

OUT, IN, RANK = 4096, 4096, 16
SCALE = 2.0
NCORES = 8
OSH = OUT // NCORES          # 512 out features per core
P = 128
KO = IN // P                 # 32 k-chunks
KQ = 4                       # k-quarters for x streaming
KO_Q = KO // KQ              # 8 k-chunks per x tile
M = 4 * 2048                 # 8192 tokens
MCH = 512                    # tokens per x tile
NM = M // MCH                # 16 m-chunks
OC = OSH // P                # 4 o-chunks of 128
NRW = 256                    # ones rhs width for norm matmuls (f32r min)

F32 = mybir.dt.float32
F32R = mybir.dt.float32r


def _build():
    nc = bacc.Bacc(None, target_bir_lowering=False)
    xT = nc.dram_tensor("xT", [P, KO, M], F32R, kind="ExternalInput")
    wT = nc.dram_tensor("wT", [P, KO, OSH], F32, kind="ExternalInput")
    aT = nc.dram_tensor("aT", [RANK, IN], F32R, kind="ExternalInput")
    bT = nc.dram_tensor("bT", [RANK, OSH], F32, kind="ExternalInput")
    wm = nc.dram_tensor("wm", [P, OC], F32, kind="ExternalInput")
    bc = nc.dram_tensor("bc", [P, OC], F32, kind="ExternalInput")
    outT = nc.dram_tensor("outT", [OSH, M], F32, kind="ExternalOutput")
    outT_v = outT.ap().rearrange("(oc p) m -> oc p m", p=P)

    with tile.TileContext(nc) as tc:
        with (
            tc.tile_pool(name="wr", bufs=1) as wrpool,
            tc.tile_pool(name="const", bufs=1) as cpool,
            tc.tile_pool(name="wv", bufs=2) as wvpool,
            tc.tile_pool(name="ach", bufs=4) as apool,
            tc.tile_pool(name="sq", bufs=2) as sqpool,
            tc.tile_pool(name="xs", bufs=4) as xpool,
            tc.tile_pool(name="os", bufs=3) as opool,
            tc.tile_pool(name="ps_mm", bufs=8, space="PSUM") as ps_mm,
        ):
            # ---- constants ----
            bt_f = cpool.tile([RANK, OSH], F32)
            nc.sync.dma_start(bt_f[:], bT.ap())
            bt2 = cpool.tile([RANK, OSH], F32R)
            nc.vector.tensor_scalar_mul(bt2[:], bt_f[:], SCALE)
            # all-ones lhsT for the norm matmuls. NOTE: f32r/f32 matmuls
            # whose PSUM output has fewer than 128 partitions compile but
            # produce a NEFF the runtime refuses to load - keep the lhsT
            # free dim (= output partitions) at 128.
            ones_f = cpool.tile([P, P], F32)
            nc.any.memset(ones_f[:], 1.0)
            ones_r = cpool.tile([P, P], F32R)
            nc.vector.tensor_copy(ones_r[:], ones_f[:])
            ident = cpool.tile([P, P], F32)
            make_identity(nc, ident)
            wm_col = cpool.tile([P, OC], F32)
            nc.sync.dma_start(wm_col[:], wm.ap())
            bias_col = cpool.tile([P, OC], F32)
            nc.sync.dma_start(bias_col[:], bc.ap())

            # ---- DoRA weight prep:
            # wr[:, ko] = W^T chunk + (2 B A)^T chunk   (PE + DVE add) ----
            wr = wrpool.tile([P, KO, OSH], F32R)
            achs = []
            for kq in range(KQ):
                ach = apool.tile([RANK, KO_Q * P], F32R, name="ach")
                nc.sync.dma_start(
                    ach[:], aT.ap()[:, kq * KO_Q * P:(kq + 1) * KO_Q * P])
                achs.append(ach)
            # nr shares the "mm" tag (same bank size): the pool's 8 banks
            # cover prep (ba cycling + nr) and the main loop's 4 live
            # accumulation groups double-buffered across m-chunks
            nr = ps_mm.tile([P, OSH], F32, name="mm")
            for kq in range(KQ):
                wv = wvpool.tile([P, KO_Q, OSH], F32)
                for k8 in range(KO_Q):
                    nc.sync.dma_start(
                        wv[:, k8], wT.ap()[:, kq * KO_Q + k8])
                for k8 in range(KO_Q):
                    ko = kq * KO_Q + k8
                    ba = ps_mm.tile([P, OSH], F32, name="mm")
                    nc.tensor.matmul(
                        ba[:], achs[kq][:, k8 * P:(k8 + 1) * P], bt2[:],
                        start=True, stop=True)
                    nc.vector.tensor_tensor(
                        wr[:, ko], wv[:, k8], ba[:], mybir.AluOpType.add)
                    # row-norm accumulation: norm2 row = ones^T @ wr^2,
                    # a single 32-matmul group in its own PSUM bank
                    # (concurrent groups must never share a bank -
                    # start=True clears whole-bank has_written state)
                    sq = sqpool.tile([P, OSH], F32R)
                    nc.vector.tensor_tensor(
                        sq[:], wr[:, ko], wr[:, ko], mybir.AluOpType.mult)
                    nc.tensor.matmul(
                        nr[:], ones_r[:], sq[:],
                        start=(ko == 0), stop=(ko == KO - 1))

            # ---- scale_col = wm / sqrt(norm2): every row of nr holds the
            # same 512 norms; PE-transpose 128-wide chunks of row space to
            # land them on o-partitions (no DRAM bounce - that reshape
            # path yields a NEFF the runtime refuses to load) ----
            sqc = cpool.tile([P, OC], F32)
            for oc in range(OC):
                nr_sb = sqpool.tile([P, P], F32, name="nrb")
                nc.vector.tensor_copy(nr_sb[:], nr[:, oc * P:(oc + 1) * P])
                pt = ps_mm.tile([P, P], F32, name="mm")
                nc.tensor.transpose(pt[:], nr_sb[:], ident[:])
                nc.scalar.activation(
                    sqc[:, oc:oc + 1], pt[:, 0:1],
                    mybir.ActivationFunctionType.Sqrt)
            rcp = cpool.tile([P, OC], F32)
            nc.vector.reciprocal(rcp[:], sqc[:])
            scale_col = cpool.tile([P, OC], F32)
            nc.vector.tensor_tensor(
                scale_col[:], wm_col[:], rcp[:], mybir.AluOpType.mult)

            # ---- main matmul: outT[o, m] accumulated over k ----
            for mc in range(NM):
                pss = [ps_mm.tile([P, MCH], F32, name="mm")
                       for _ in range(OC)]
                for kq in range(KQ):
                    xt = xpool.tile([P, KO_Q, MCH], F32R)
                    nc.sync.dma_start(
                        xt[:],
                        xT.ap()[:, kq * KO_Q:(kq + 1) * KO_Q,
                                mc * MCH:(mc + 1) * MCH])
                    for oc in range(OC):
                        for k8 in range(KO_Q):
                            nc.tensor.matmul(
                                pss[oc][:],
                                wr[:, kq * KO_Q + k8, oc * P:(oc + 1) * P],
                                xt[:, k8],
                                start=(kq == 0 and k8 == 0),
                                stop=(kq == KQ - 1 and k8 == KO_Q - 1))
                for oc in range(OC):
                    ot = opool.tile([P, MCH], F32)
                    nc.vector.tensor_scalar(
                        ot[:], pss[oc][:],
                        scale_col[:, oc:oc + 1], bias_col[:, oc:oc + 1],
                        mybir.AluOpType.mult, mybir.AluOpType.add)
                    # gpsimd (SWDGE) queue: keeps output writes off the
                    # sync queue that streams the x tiles
                    nc.gpsimd.dma_start(
                        outT_v[oc, :, mc * MCH:(mc + 1) * MCH], ot[:])
    nc.compile()
    return nc


def kernel(x, base_weight, base_bias, weight_m, dora_A, dora_B):
    x = np.asarray(x, dtype=np.float32)
    base_weight = np.asarray(base_weight, dtype=np.float32)
    base_bias = np.asarray(base_bias, dtype=np.float32)
    weight_m = np.asarray(weight_m, dtype=np.float32)
    dora_A = np.asarray(dora_A, dtype=np.float32)
    dora_B = np.asarray(dora_B, dtype=np.float32)

    B, S, _ = x.shape
    assert B * S == M and x.shape[2] == IN

    # xT[p, ko, m] = x[m, ko*128+p]  (shared across all cores)
    x2 = x.reshape(M, KO, P)
    xT = np.ascontiguousarray(x2.transpose(2, 1, 0))

    in_maps = []
    for c in range(NCORES):
        sl = slice(c * OSH, (c + 1) * OSH)
        w_c = base_weight[sl]                                   # [OSH, IN]
        wT_c = np.ascontiguousarray(
            w_c.reshape(OSH, KO, P).transpose(2, 1, 0))         # [P, KO, OSH]
        bT_c = np.ascontiguousarray(dora_B[sl].T)               # [RANK, OSH]
        wm_c = np.ascontiguousarray(weight_m[sl].reshape(OC, P).T)
        bc_c = np.ascontiguousarray(base_bias[sl].reshape(OC, P).T)
        in_maps.append({
            "xT": xT,
            "wT": wT_c,
            "aT": dora_A,
            "bT": bT_c,
            "wm": wm_c,
            "bc": bc_c,
        })

    nc = _build()
    res = run_bass_kernel_spmd(nc, in_maps, core_ids=list(range(NCORES)))

    full = np.empty((OUT, M), dtype=np.float32)
    for c in range(NCORES):
        full[c * OSH:(c + 1) * OSH] = res.results[c]["outT"]
    return np.ascontiguousarray(full.T).reshape(B, S, OUT)


# revision 55
# speedup vs baseline: 1.0397x; 1.0397x over previous
"""DoRA Linear on 8 Trainium2 NeuronCores (Bass/Tile).

Reference computation (all fp32):
    new_v   = base_weight + SCALE * dora_B @ dora_A          [OUT, IN]
    scale_o = weight_m / ||new_v||_row                        [OUT]
    out     = x @ (scale_o[:, None] * new_v)^T + base_bias    [B, S, OUT]

Sharding: column-parallel over OUT across 8 cores (OUT/8 = 512 each).
base_weight, dora_B, weight_m, base_bias sharded; x, dora_A replicated.

Per-core device program (heavy math in float32r on the PE):
  1. Build W'^T = (W + SCALE*B@A)^T chunk-by-chunk in SBUF:
     PE matmul A^T@(2B^T) -> PSUM, DVE adds the W^T chunk and writes the
     f32r weight tile `wr` the main matmul consumes.
  2. Row norms: sq = wr*wr (DVE), then PE matmuls with sq as lhsT and an
     all-ones rhs (N=256) accumulate sum_i sq[i, o] over k-chunks,
     giving norms directly in o-partition column layout.  NOTE: f32/f32r
     matmuls whose PSUM output has fewer than 128 partitions compile but
     produce a NEFF the runtime refuses to load - keep M = 128.
  3. scale_col = weight_m / sqrt(norm2)  (ACT sqrt + DVE reciprocal/mul)
  4. Main matmul, output-transposed orientation:
     outT[o, m] = sum_k W'^T[k, o] * xT[k, m], PSUM-accumulated over 32
     k-chunks; eviction fuses *scale_o + bias_o in one DVE tensor_scalar
     (per-partition scalars).
Host: layout transposes in numpy (no FLOPs), final gather + transpose.
"""

import numpy as np

import concourse.mybir as mybir
import concourse.tile as tile
from concourse import bacc
from concourse.bass_utils import run_bass_kernel_spmd
from concourse.masks import make_identity

OUT, IN, RANK = 4096, 4096, 16
SCALE = 2.0
NCORES = 8
OSH = OUT // NCORES          # 512 out features per core
P = 128
KO = IN // P                 # 32 k-chunks
KQ = 4                       # k-quarters for x streaming
KO_Q = KO // KQ              # 8 k-chunks per x tile
M = 4 * 2048                 # 8192 tokens
MCH = 512                    # tokens per x tile
NM = M // MCH                # 16 m-chunks
OC = OSH // P                # 4 o-chunks of 128
NRW = 256                    # ones rhs width for norm matmuls (f32r min)

F32 = mybir.dt.float32
F32R = mybir.dt.float32r


def _build():
    nc = bacc.Bacc(None, target_bir_lowering=False)
    xT = nc.dram_tensor("xT", [P, KO, M], F32R, kind="ExternalInput")
    wT = nc.dram_tensor("wT", [P, KO, OSH], F32, kind="ExternalInput")
    aT = nc.dram_tensor("aT", [RANK, IN], F32R, kind="ExternalInput")
    bT = nc.dram_tensor("bT", [RANK, OSH], F32, kind="ExternalInput")
    wm = nc.dram_tensor("wm", [P, OC], F32, kind="ExternalInput")
    bc = nc.dram_tensor("bc", [P, OC], F32, kind="ExternalInput")
    outT = nc.dram_tensor("outT", [OSH, M], F32, kind="ExternalOutput")
    outT_v = outT.ap().rearrange("(oc p) m -> oc p m", p=P)

    with tile.TileContext(nc) as tc:
        with (
            tc.tile_pool(name="wr", bufs=1) as wrpool,
            tc.tile_pool(name="const", bufs=1) as cpool,
            tc.tile_pool(name="wv", bufs=1) as wvpool,
            tc.tile_pool(name="ach", bufs=4) as apool,
            tc.tile_pool(name="sq", bufs=2) as sqpool,
            tc.tile_pool(name="xs", bufs=5) as xpool,
            tc.tile_pool(name="os", bufs=3) as opool,
            tc.tile_pool(name="ps_mm", bufs=8, space="PSUM") as ps_mm,
        ):
            # ---- constants ----
            bt_f = cpool.tile([RANK, OSH], F32)
            nc.sync.dma_start(bt_f[:], bT.ap())
            bt2 = cpool.tile([RANK, OSH], F32R)
            nc.vector.tensor_scalar_mul(bt2[:], bt_f[:], SCALE)
            # all-ones lhsT for the norm matmuls. NOTE: f32r/f32 matmuls
            # whose PSUM output has fewer than 128 partitions compile but
            # produce a NEFF the runtime refuses to load - keep the lhsT
            # free dim (= output partitions) at 128.
            ones_f = cpool.tile([P, P], F32)
            nc.any.memset(ones_f[:], 1.0)
            ones_r = cpool.tile([P, P], F32R)
            nc.vector.tensor_copy(ones_r[:], ones_f[:])
            ident = cpool.tile([P, P], F32)
            make_identity(nc, ident)
            wm_col = cpool.tile([P, OC], F32)
            nc.sync.dma_start(wm_col[:], wm.ap())
            bias_col = cpool.tile([P, OC], F32)
            nc.sync.dma_start(bias_col[:], bc.ap())

            # ---- DoRA weight prep:
            # wr[:, ko] = W^T chunk + (2 B A)^T chunk   (PE + DVE add) ----
            wr = wrpool.tile([P, KO, OSH], F32R)
            achs = []
            for kq in range(KQ):
                ach = apool.tile([RANK, KO_Q * P], F32R, name="ach")
                nc.sync.dma_start(
                    ach[:], aT.ap()[:, kq * KO_Q * P:(kq + 1) * KO_Q * P])
                achs.append(ach)
            # nr shares the "mm" tag (same bank size): the pool's 8 banks
            # cover prep (ba cycling + nr) and the main loop's 4 live
            # accumulation groups double-buffered across m-chunks
            nr = ps_mm.tile([P, OSH], F32, name="mm")
            for kq in range(KQ):
                wv = wvpool.tile([P, KO_Q, OSH], F32)
                for k8 in range(KO_Q):
                    nc.sync.dma_start(
                        wv[:, k8], wT.ap()[:, kq * KO_Q + k8])
                for k8 in range(KO_Q):
                    ko = kq * KO_Q + k8
                    ba = ps_mm.tile([P, OSH], F32, name="mm")
                    nc.tensor.matmul(
                        ba[:], achs[kq][:, k8 * P:(k8 + 1) * P], bt2[:],
                        start=True, stop=True)
                    nc.vector.tensor_tensor(
                        wr[:, ko], wv[:, k8], ba[:], mybir.AluOpType.add)
                    # row-norm accumulation: norm2 row = ones^T @ wr^2,
                    # a single 32-matmul group in its own PSUM bank
                    # (concurrent groups must never share a bank -
                    # start=True clears whole-bank has_written state)
                    sq = sqpool.tile([P, OSH], F32R)
                    nc.vector.tensor_tensor(
                        sq[:], wr[:, ko], wr[:, ko], mybir.AluOpType.mult)
                    nc.tensor.matmul(
                        nr[:], ones_r[:], sq[:],
                        start=(ko == 0), stop=(ko == KO - 1))

            # ---- scale_col = wm / sqrt(norm2): every row of nr holds the
            # same 512 norms; PE-transpose 128-wide chunks of row space to
            # land them on o-partitions (no DRAM bounce - that reshape
            # path yields a NEFF the runtime refuses to load) ----
            sqc = cpool.tile([P, OC], F32)
            for oc in range(OC):
                nr_sb = sqpool.tile([P, P], F32, name="nrb")
                nc.vector.tensor_copy(nr_sb[:], nr[:, oc * P:(oc + 1) * P])
                pt = ps_mm.tile([P, P], F32, name="mm")
                nc.tensor.transpose(pt[:], nr_sb[:], ident[:])
                nc.scalar.activation(
                    sqc[:, oc:oc + 1], pt[:, 0:1],
                    mybir.ActivationFunctionType.Sqrt)
            rcp = cpool.tile([P, OC], F32)
            nc.vector.reciprocal(rcp[:], sqc[:])
            scale_col = cpool.tile([P, OC], F32)
            nc.vector.tensor_tensor(
                scale_col[:], wm_col[:], rcp[:], mybir.AluOpType.mult)

            # ---- main matmul: outT[o, m] accumulated over k ----
            for mc in range(NM):
                pss = [ps_mm.tile([P, MCH], F32, name="mm")
                       for _ in range(OC)]
                for kq in range(KQ):
                    xt = xpool.tile([P, KO_Q, MCH], F32R)
                    nc.sync.dma_start(
                        xt[:],
                        xT.ap()[:, kq * KO_Q:(kq + 1) * KO_Q,
                                mc * MCH:(mc + 1) * MCH])
                    for oc in range(OC):
                        for k8 in range(KO_Q):
                            nc.tensor.matmul(
                                pss[oc][:],
                                wr[:, kq * KO_Q + k8, oc * P:(oc + 1) * P],
                                xt[:, k8],
                                start=(kq == 0 and k8 == 0),
                                stop=(kq == KQ - 1 and k8 == KO_Q - 1))
                for oc in range(OC):
                    ot = opool.tile([P, MCH], F32)
                    nc.vector.tensor_scalar(
                        ot[:], pss[oc][:],
                        scale_col[:, oc:oc + 1], bias_col[:, oc:oc + 1],
                        mybir.AluOpType.mult, mybir.AluOpType.add)
                    # gpsimd (SWDGE) queue: keeps output writes off the
                    # sync queue that streams the x tiles
                    nc.gpsimd.dma_start(
                        outT_v[oc, :, mc * MCH:(mc + 1) * MCH], ot[:])
    nc.compile()
    return nc


def kernel(x, base_weight, base_bias, weight_m, dora_A, dora_B):
    x = np.asarray(x, dtype=np.float32)
    base_weight = np.asarray(base_weight, dtype=np.float32)
    base_bias = np.asarray(base_bias, dtype=np.float32)
    weight_m = np.asarray(weight_m, dtype=np.float32)
    dora_A = np.asarray(dora_A, dtype=np.float32)
    dora_B = np.asarray(dora_B, dtype=np.float32)

    B, S, _ = x.shape
    assert B * S == M and x.shape[2] == IN

    # xT[p, ko, m] = x[m, ko*128+p]  (shared across all cores)
    x2 = x.reshape(M, KO, P)
    xT = np.ascontiguousarray(x2.transpose(2, 1, 0))

    in_maps = []
    for c in range(NCORES):
        sl = slice(c * OSH, (c + 1) * OSH)
        w_c = base_weight[sl]                                   # [OSH, IN]
        wT_c = np.ascontiguousarray(
            w_c.reshape(OSH, KO, P).transpose(2, 1, 0))         # [P, KO, OSH]
        bT_c = np.ascontiguousarray(dora_B[sl].T)               # [RANK, OSH]
        wm_c = np.ascontiguousarray(weight_m[sl].reshape(OC, P).T)
        bc_c = np.ascontiguousarray(base_bias[sl].reshape(OC, P).T)
        in_maps.append({
            "xT": xT,
            "wT": wT_c,
            "aT": dora_A,
            "bT": bT_c,
            "wm": wm_c,
            "bc": bc_c,
        })

    nc = _build()
    res = run_bass_kernel_spmd(nc, in_maps, core_ids=list(range(NCORES)))

    full = np.empty((OUT, M), dtype=np.float32)
    for c in range(NCORES):
        full[c * OSH:(c + 1) * OSH] = res.results[c]["outT"]
    return np.ascontiguousarray(full.T).reshape(B, S, OUT)


# revision 57
# speedup vs baseline: 1.0686x; 1.0278x over previous
"""DoRA Linear on 8 Trainium2 NeuronCores (Bass/Tile).

Reference computation (all fp32):
    new_v   = base_weight + SCALE * dora_B @ dora_A          [OUT, IN]
    scale_o = weight_m / ||new_v||_row                        [OUT]
    out     = x @ (scale_o[:, None] * new_v)^T + base_bias    [B, S, OUT]

Sharding: column-parallel over OUT across 8 cores (OUT/8 = 512 each).
base_weight, dora_B, weight_m, base_bias sharded; x, dora_A replicated.

Per-core device program (heavy math in float32r on the PE):
  1. Build W'^T = (W + SCALE*B@A)^T chunk-by-chunk in SBUF:
     PE matmul A^T@(2B^T) -> PSUM, DVE adds the W^T chunk and writes the
     f32r weight tile `wr` the main matmul consumes.
  2. Row norms: sq = wr*wr (DVE), then PE matmuls with sq as lhsT and an
     all-ones rhs (N=256) accumulate sum_i sq[i, o] over k-chunks,
     giving norms directly in o-partition column layout.  NOTE: f32/f32r
     matmuls whose PSUM output has fewer than 128 partitions compile but
     produce a NEFF the runtime refuses to load - keep M = 128.
  3. scale_col = weight_m / sqrt(norm2)  (ACT sqrt + DVE reciprocal/mul)
  4. Main matmul, output-transposed orientation:
     outT[o, m] = sum_k W'^T[k, o] * xT[k, m], PSUM-accumulated over 32
     k-chunks; eviction fuses *scale_o + bias_o in one DVE tensor_scalar
     (per-partition scalars).
Host: layout transposes in numpy (no FLOPs), final gather + transpose.
"""

import numpy as np

import concourse.mybir as mybir
import concourse.tile as tile
from concourse import bacc
from concourse.bass_utils import run_bass_kernel_spmd
from concourse.masks import make_identity

OUT, IN, RANK = 4096, 4096, 16
SCALE = 2.0
NCORES = 8
OSH = OUT // NCORES          # 512 out features per core
P = 128
KO = IN // P                 # 32 k-chunks
KQ = 4                       # k-quarters for x streaming
KO_Q = KO // KQ              # 8 k-chunks per x tile
M = 4 * 2048                 # 8192 tokens
MCH = 512                    # tokens per x tile
NM = M // MCH                # 16 m-chunks
OC = OSH // P                # 4 o-chunks of 128
NRW = 256                    # ones rhs width for norm matmuls (f32r min)

F32 = mybir.dt.float32
F32R = mybir.dt.float32r


def _build():
    nc = bacc.Bacc(None, target_bir_lowering=False)
    xT = nc.dram_tensor("xT", [P, KO, M], F32R, kind="ExternalInput")
    wT = nc.dram_tensor("wT", [P, KO, OSH], F32, kind="ExternalInput")
    aT = nc.dram_tensor("aT", [RANK, IN], F32R, kind="ExternalInput")
    bT = nc.dram_tensor("bT", [RANK, OSH], F32, kind="ExternalInput")
    wm = nc.dram_tensor("wm", [P, OC], F32, kind="ExternalInput")
    bc = nc.dram_tensor("bc", [P, OC], F32, kind="ExternalInput")
    outT = nc.dram_tensor("outT", [OSH, M], F32, kind="ExternalOutput")
    outT_v = outT.ap().rearrange("(oc p) m -> oc p m", p=P)

    with tile.TileContext(nc) as tc:
        with (
            tc.tile_pool(name="wr", bufs=1) as wrpool,
            tc.tile_pool(name="const", bufs=1) as cpool,
            tc.tile_pool(name="wv", bufs=2) as wvpool,
            tc.tile_pool(name="ach", bufs=4) as apool,
            tc.tile_pool(name="sq", bufs=2) as sqpool,
            tc.tile_pool(name="xs", bufs=4) as xpool,
            tc.tile_pool(name="os", bufs=3) as opool,
            tc.tile_pool(name="ps_mm", bufs=8, space="PSUM") as ps_mm,
        ):
            # ---- constants ----
            bt_f = cpool.tile([RANK, OSH], F32)
            nc.sync.dma_start(bt_f[:], bT.ap())
            bt2 = cpool.tile([RANK, OSH], F32R)
            nc.vector.tensor_scalar_mul(bt2[:], bt_f[:], SCALE)
            # all-ones lhsT for the norm matmuls. NOTE: f32r/f32 matmuls
            # whose PSUM output has fewer than 128 partitions compile but
            # produce a NEFF the runtime refuses to load - keep the lhsT
            # free dim (= output partitions) at 128.
            ones_f = cpool.tile([P, P], F32)
            nc.any.memset(ones_f[:], 1.0)
            ones_r = cpool.tile([P, P], F32R)
            nc.vector.tensor_copy(ones_r[:], ones_f[:])
            ident = cpool.tile([P, P], F32)
            make_identity(nc, ident)
            wm_col = cpool.tile([P, OC], F32)
            nc.sync.dma_start(wm_col[:], wm.ap())
            bias_col = cpool.tile([P, OC], F32)
            nc.sync.dma_start(bias_col[:], bc.ap())

            # ---- DoRA weight prep:
            # wr[:, ko] = W^T chunk + (2 B A)^T chunk   (PE + DVE add) ----
            wr = wrpool.tile([P, KO, OSH], F32R)
            achs = []
            for kq in range(KQ):
                ach = apool.tile([RANK, KO_Q * P], F32R, name="ach")
                nc.sync.dma_start(
                    ach[:], aT.ap()[:, kq * KO_Q * P:(kq + 1) * KO_Q * P])
                achs.append(ach)
            # nr shares the "mm" tag (same bank size): the pool's 8 banks
            # cover prep (ba cycling + nr) and the main loop's 4 live
            # accumulation groups double-buffered across m-chunks
            nr = ps_mm.tile([P, OSH], F32, name="mm")
            for kq in range(KQ):
                wv = wvpool.tile([P, KO_Q, OSH], F32)
                for k8 in range(KO_Q):
                    nc.sync.dma_start(
                        wv[:, k8], wT.ap()[:, kq * KO_Q + k8])
                for k8 in range(KO_Q):
                    ko = kq * KO_Q + k8
                    ba = ps_mm.tile([P, OSH], F32, name="mm")
                    nc.tensor.matmul(
                        ba[:], achs[kq][:, k8 * P:(k8 + 1) * P], bt2[:],
                        start=True, stop=True)
                    nc.vector.tensor_tensor(
                        wr[:, ko], wv[:, k8], ba[:], mybir.AluOpType.add)
                    # row-norm accumulation: norm2 row = ones^T @ wr^2,
                    # a single 32-matmul group in its own PSUM bank
                    # (concurrent groups must never share a bank -
                    # start=True clears whole-bank has_written state)
                    sq = sqpool.tile([P, OSH], F32R)
                    nc.vector.tensor_tensor(
                        sq[:], wr[:, ko], wr[:, ko], mybir.AluOpType.mult)
                    nc.tensor.matmul(
                        nr[:], ones_r[:], sq[:],
                        start=(ko == 0), stop=(ko == KO - 1))

            # ---- scale_col = wm / sqrt(norm2): every row of nr holds the
            # same 512 norms; PE-transpose 128-wide chunks of row space to
            # land them on o-partitions (no DRAM bounce - that reshape
            # path yields a NEFF the runtime refuses to load) ----
            sqc = cpool.tile([P, OC], F32)
            for oc in range(OC):
                nr_sb = sqpool.tile([P, P], F32, name="nrb")
                nc.vector.tensor_copy(nr_sb[:], nr[:, oc * P:(oc + 1) * P])
                pt = ps_mm.tile([P, P], F32, name="mm")
                nc.tensor.transpose(pt[:], nr_sb[:], ident[:])
                nc.scalar.activation(
                    sqc[:, oc:oc + 1], pt[:, 0:1],
                    mybir.ActivationFunctionType.Sqrt)
            rcp = cpool.tile([P, OC], F32)
            nc.vector.reciprocal(rcp[:], sqc[:])
            scale_col = cpool.tile([P, OC], F32)
            nc.vector.tensor_tensor(
                scale_col[:], wm_col[:], rcp[:], mybir.AluOpType.mult)

            # ---- main matmul: outT[o, m] accumulated over k ----
            for mc in range(NM):
                pss = [ps_mm.tile([P, MCH], F32, name="mm")
                       for _ in range(OC)]
                for kq in range(KQ):
                    xt = xpool.tile([P, KO_Q, MCH], F32R)
                    nc.sync.dma_start(
                        xt[:],
                        xT.ap()[:, kq * KO_Q:(kq + 1) * KO_Q,
                                mc * MCH:(mc + 1) * MCH])
                    for oc in range(OC):
                        for k8 in range(KO_Q):
                            nc.tensor.matmul(
                                pss[oc][:],
                                wr[:, kq * KO_Q + k8, oc * P:(oc + 1) * P],
                                xt[:, k8],
                                start=(kq == 0 and k8 == 0),
                                stop=(kq == KQ - 1 and k8 == KO_Q - 1))
                for oc in range(OC):
                    ot = opool.tile([P, MCH], F32)
                    nc.vector.tensor_scalar(
                        ot[:], pss[oc][:],
                        scale_col[:, oc:oc + 1], bias_col[:, oc:oc + 1],
                        mybir.AluOpType.mult, mybir.AluOpType.add)
                    # gpsimd (SWDGE) queue: keeps output writes off the
                    # sync queue that streams the x tiles
                    nc.gpsimd.dma_start(
                        outT_v[oc, :, mc * MCH:(mc + 1) * MCH], ot[:])
    nc.compile()
    return nc


def kernel(x, base_weight, base_bias, weight_m, dora_A, dora_B):
    x = np.asarray(x, dtype=np.float32)
    base_weight = np.asarray(base_weight, dtype=np.float32)
    base_bias = np.asarray(base_bias, dtype=np.float32)
    weight_m = np.asarray(weight_m, dtype=np.float32)
    dora_A = np.asarray(dora_A, dtype=np.float32)
    dora_B = np.asarray(dora_B, dtype=np.float32)

    B, S, _ = x.shape
    assert B * S == M and x.shape[2] == IN

    # xT[p, ko, m] = x[m, ko*128+p]  (shared across all cores)
    x2 = x.reshape(M, KO, P)
    xT = np.ascontiguousarray(x2.transpose(2, 1, 0))

    in_maps = []
    for c in range(NCORES):
        sl = slice(c * OSH, (c + 1) * OSH)
        w_c = base_weight[sl]                                   # [OSH, IN]
        wT_c = np.ascontiguousarray(
            w_c.reshape(OSH, KO, P).transpose(2, 1, 0))         # [P, KO, OSH]
        bT_c = np.ascontiguousarray(dora_B[sl].T)               # [RANK, OSH]
        wm_c = np.ascontiguousarray(weight_m[sl].reshape(OC, P).T)
        bc_c = np.ascontiguousarray(base_bias[sl].reshape(OC, P).T)
        in_maps.append({
            "xT": xT,
            "wT": wT_c,
            "aT": dora_A,
            "bT": bT_c,
            "wm": wm_c,
            "bc": bc_c,
        })

    nc = _build()
    res = run_bass_kernel_spmd(nc, in_maps, core_ids=list(range(NCORES)))

    full = np.empty((OUT, M), dtype=np.float32)
    for c in range(NCORES):
        full[c * OSH:(c + 1) * OSH] = res.results[c]["outT"]
    return np.ascontiguousarray(full.T).reshape(B, S, OUT)


# revision 62
# speedup vs baseline: 1.1480x; 1.0743x over previous
"""DoRA Linear on 8 Trainium2 NeuronCores (Bass/Tile).

Reference computation (all fp32):
    new_v   = base_weight + SCALE * dora_B @ dora_A          [OUT, IN]
    scale_o = weight_m / ||new_v||_row                        [OUT]
    out     = x @ (scale_o[:, None] * new_v)^T + base_bias    [B, S, OUT]

Sharding: column-parallel over OUT across 8 cores (OUT/8 = 512 each).
base_weight, dora_B, weight_m, base_bias sharded; x, dora_A replicated.

Per-core device program (heavy math in float32r on the PE):
  1. Build W'^T = (W + SCALE*B@A)^T chunk-by-chunk in SBUF:
     PE matmul A^T@(2B^T) -> PSUM, DVE adds the W^T chunk and writes the
     f32r weight tile `wr` the main matmul consumes.
  2. Row norms: sq = wr*wr (DVE), then PE matmuls with sq as lhsT and an
     all-ones rhs (N=256) accumulate sum_i sq[i, o] over k-chunks,
     giving norms directly in o-partition column layout.  NOTE: f32/f32r
     matmuls whose PSUM output has fewer than 128 partitions compile but
     produce a NEFF the runtime refuses to load - keep M = 128.
  3. scale_col = weight_m / sqrt(norm2)  (ACT sqrt + DVE reciprocal/mul)
  4. Main matmul, output-transposed orientation:
     outT[o, m] = sum_k W'^T[k, o] * xT[k, m], PSUM-accumulated over 32
     k-chunks; eviction fuses *scale_o + bias_o in one DVE tensor_scalar
     (per-partition scalars).
Host: layout transposes in numpy (no FLOPs), final gather + transpose.
"""

import numpy as np

import concourse.mybir as mybir
import concourse.tile as tile
from concourse import bacc
from concourse.bass_utils import run_bass_kernel_spmd
from concourse.masks import make_identity

OUT, IN, RANK = 4096, 4096, 16
SCALE = 2.0
NCORES = 8
OSH = OUT // NCORES          # 512 out features per core
P = 128
KO = IN // P                 # 32 k-chunks
KQ = 4                       # k-quarters for x streaming
KO_Q = KO // KQ              # 8 k-chunks per x tile
M = 4 * 2048                 # 8192 tokens
MCH = 512                    # tokens per x tile
NM = M // MCH                # 16 m-chunks
OC = OSH // P                # 4 o-chunks of 128
NRW = 256                    # ones rhs width for norm matmuls (f32r min)

F32 = mybir.dt.float32
F32R = mybir.dt.float32r
F16 = mybir.dt.float16


def _build():
    nc = bacc.Bacc(None, target_bir_lowering=False)
    xT = nc.dram_tensor("xT", [P, KO, M], F16, kind="ExternalInput")
    wT = nc.dram_tensor("wT", [P, KO, OSH], F32, kind="ExternalInput")
    aT = nc.dram_tensor("aT", [RANK, IN], F32R, kind="ExternalInput")
    bT = nc.dram_tensor("bT", [RANK, OSH], F32, kind="ExternalInput")
    wm = nc.dram_tensor("wm", [P, OC], F32, kind="ExternalInput")
    bc = nc.dram_tensor("bc", [P, OC], F32, kind="ExternalInput")
    outT = nc.dram_tensor("outT", [OSH, M], F32, kind="ExternalOutput")
    outT_v = outT.ap().rearrange("(oc p) m -> oc p m", p=P)

    with tile.TileContext(nc) as tc:
        with (
            tc.tile_pool(name="wr", bufs=1) as wrpool,
            tc.tile_pool(name="const", bufs=1) as cpool,
            tc.tile_pool(name="wv", bufs=2) as wvpool,
            tc.tile_pool(name="ach", bufs=4) as apool,
            tc.tile_pool(name="sq", bufs=2) as sqpool,
            tc.tile_pool(name="xs", bufs=4) as xpool,
            tc.tile_pool(name="os", bufs=3) as opool,
            tc.tile_pool(name="ps_mm", bufs=8, space="PSUM") as ps_mm,
        ):
            # ---- constants ----
            bt_f = cpool.tile([RANK, OSH], F32)
            nc.sync.dma_start(bt_f[:], bT.ap())
            bt2 = cpool.tile([RANK, OSH], F32R)
            nc.vector.tensor_scalar_mul(bt2[:], bt_f[:], SCALE)
            # all-ones lhsT for the norm matmuls. NOTE: f32r/f32 matmuls
            # whose PSUM output has fewer than 128 partitions compile but
            # produce a NEFF the runtime refuses to load - keep the lhsT
            # free dim (= output partitions) at 128.
            ones_f = cpool.tile([P, P], F32)
            nc.any.memset(ones_f[:], 1.0)
            ones_r = cpool.tile([P, P], F32R)
            nc.vector.tensor_copy(ones_r[:], ones_f[:])
            ident = cpool.tile([P, P], F32)
            make_identity(nc, ident)
            wm_col = cpool.tile([P, OC], F32)
            nc.sync.dma_start(wm_col[:], wm.ap())
            bias_col = cpool.tile([P, OC], F32)
            nc.sync.dma_start(bias_col[:], bc.ap())

            # ---- DoRA weight prep:
            # wr[:, ko] = W^T chunk + (2 B A)^T chunk   (PE + DVE add) ----
            wr = wrpool.tile([P, KO, OSH], F16)
            achs = []
            for kq in range(KQ):
                ach = apool.tile([RANK, KO_Q * P], F32R, name="ach")
                nc.sync.dma_start(
                    ach[:], aT.ap()[:, kq * KO_Q * P:(kq + 1) * KO_Q * P])
                achs.append(ach)
            # nr shares the "mm" tag (same bank size): the pool's 8 banks
            # cover prep (ba cycling + nr) and the main loop's 4 live
            # accumulation groups double-buffered across m-chunks
            nr = ps_mm.tile([P, OSH], F32, name="mm")
            for kq in range(KQ):
                wv = wvpool.tile([P, KO_Q, OSH], F32)
                for k8 in range(KO_Q):
                    nc.sync.dma_start(
                        wv[:, k8], wT.ap()[:, kq * KO_Q + k8])
                for k8 in range(KO_Q):
                    ko = kq * KO_Q + k8
                    ba = ps_mm.tile([P, OSH], F32, name="mm")
                    nc.tensor.matmul(
                        ba[:], achs[kq][:, k8 * P:(k8 + 1) * P], bt2[:],
                        start=True, stop=True)
                    nc.vector.tensor_tensor(
                        wr[:, ko], wv[:, k8], ba[:], mybir.AluOpType.add)
                    # row-norm accumulation: norm2 row = ones^T @ wr^2,
                    # a single 32-matmul group in its own PSUM bank
                    # (concurrent groups must never share a bank -
                    # start=True clears whole-bank has_written state)
                    sq = sqpool.tile([P, OSH], F32R)
                    nc.vector.tensor_tensor(
                        sq[:], wr[:, ko], wr[:, ko], mybir.AluOpType.mult)
                    nc.tensor.matmul(
                        nr[:], ones_r[:], sq[:],
                        start=(ko == 0), stop=(ko == KO - 1))

            # ---- scale_col = wm / sqrt(norm2): every row of nr holds the
            # same 512 norms; PE-transpose 128-wide chunks of row space to
            # land them on o-partitions (no DRAM bounce - that reshape
            # path yields a NEFF the runtime refuses to load) ----
            sqc = cpool.tile([P, OC], F32)
            for oc in range(OC):
                nr_sb = sqpool.tile([P, P], F32, name="nrb")
                nc.vector.tensor_copy(nr_sb[:], nr[:, oc * P:(oc + 1) * P])
                pt = ps_mm.tile([P, P], F32, name="mm")
                nc.tensor.transpose(pt[:], nr_sb[:], ident[:])
                nc.scalar.activation(
                    sqc[:, oc:oc + 1], pt[:, 0:1],
                    mybir.ActivationFunctionType.Sqrt)
            rcp = cpool.tile([P, OC], F32)
            nc.vector.reciprocal(rcp[:], sqc[:])
            scale_col = cpool.tile([P, OC], F32)
            nc.vector.tensor_tensor(
                scale_col[:], wm_col[:], rcp[:], mybir.AluOpType.mult)

            # ---- main matmul: outT[o, m] accumulated over k ----
            for mc in range(NM):
                pss = [ps_mm.tile([P, MCH], F32, name="mm")
                       for _ in range(OC)]
                for kq in range(KQ):
                    xt = xpool.tile([P, KO_Q, MCH], F16)
                    nc.sync.dma_start(
                        xt[:],
                        xT.ap()[:, kq * KO_Q:(kq + 1) * KO_Q,
                                mc * MCH:(mc + 1) * MCH])
                    for oc in range(OC):
                        for k8 in range(KO_Q):
                            nc.tensor.matmul(
                                pss[oc][:],
                                wr[:, kq * KO_Q + k8, oc * P:(oc + 1) * P],
                                xt[:, k8],
                                start=(kq == 0 and k8 == 0),
                                stop=(kq == KQ - 1 and k8 == KO_Q - 1))
                for oc in range(OC):
                    ot = opool.tile([P, MCH], F32)
                    nc.vector.tensor_scalar(
                        ot[:], pss[oc][:],
                        scale_col[:, oc:oc + 1], bias_col[:, oc:oc + 1],
                        mybir.AluOpType.mult, mybir.AluOpType.add)
                    # gpsimd (SWDGE) queue: keeps output writes off the
                    # sync queue that streams the x tiles
                    nc.gpsimd.dma_start(
                        outT_v[oc, :, mc * MCH:(mc + 1) * MCH], ot[:])
    nc.compile()
    return nc


def kernel(x, base_weight, base_bias, weight_m, dora_A, dora_B):
    x = np.asarray(x, dtype=np.float32)
    base_weight = np.asarray(base_weight, dtype=np.float32)
    base_bias = np.asarray(base_bias, dtype=np.float32)
    weight_m = np.asarray(weight_m, dtype=np.float32)
    dora_A = np.asarray(dora_A, dtype=np.float32)
    dora_B = np.asarray(dora_B, dtype=np.float32)

    B, S, _ = x.shape
    assert B * S == M and x.shape[2] == IN

    # xT[p, ko, m] = x[m, ko*128+p]  (fp16, shared across all cores)
    x2 = x.reshape(M, KO, P)
    xT = np.ascontiguousarray(x2.transpose(2, 1, 0)).astype(np.float16)

    in_maps = []
    for c in range(NCORES):
        sl = slice(c * OSH, (c + 1) * OSH)
        w_c = base_weight[sl]                                   # [OSH, IN]
        wT_c = np.ascontiguousarray(
            w_c.reshape(OSH, KO, P).transpose(2, 1, 0))         # [P, KO, OSH]
        bT_c = np.ascontiguousarray(dora_B[sl].T)               # [RANK, OSH]
        wm_c = np.ascontiguousarray(weight_m[sl].reshape(OC, P).T)
        bc_c = np.ascontiguousarray(base_bias[sl].reshape(OC, P).T)
        in_maps.append({
            "xT": xT,
            "wT": wT_c,
            "aT": dora_A,
            "bT": bT_c,
            "wm": wm_c,
            "bc": bc_c,
        })

    nc = _build()
    res = run_bass_kernel_spmd(nc, in_maps, core_ids=list(range(NCORES)))

    full = np.empty((OUT, M), dtype=np.float32)
    for c in range(NCORES):
        full[c * OSH:(c + 1) * OSH] = res.results[c]["outT"]
    return np.ascontiguousarray(full.T).reshape(B, S, OUT)


# revision 64
# speedup vs baseline: 1.1779x; 1.0261x over previous
"""DoRA Linear on 8 Trainium2 NeuronCores (Bass/Tile).

Reference computation (all fp32):
    new_v   = base_weight + SCALE * dora_B @ dora_A          [OUT, IN]
    scale_o = weight_m / ||new_v||_row                        [OUT]
    out     = x @ (scale_o[:, None] * new_v)^T + base_bias    [B, S, OUT]

Sharding: column-parallel over OUT across 8 cores (OUT/8 = 512 each).
base_weight, dora_B, weight_m, base_bias sharded; x, dora_A replicated.

Per-core device program (heavy math in float32r on the PE):
  1. Build W'^T = (W + SCALE*B@A)^T chunk-by-chunk in SBUF:
     PE matmul A^T@(2B^T) -> PSUM, DVE adds the W^T chunk and writes the
     f32r weight tile `wr` the main matmul consumes.
  2. Row norms: sq = wr*wr (DVE), then PE matmuls with sq as lhsT and an
     all-ones rhs (N=256) accumulate sum_i sq[i, o] over k-chunks,
     giving norms directly in o-partition column layout.  NOTE: f32/f32r
     matmuls whose PSUM output has fewer than 128 partitions compile but
     produce a NEFF the runtime refuses to load - keep M = 128.
  3. scale_col = weight_m / sqrt(norm2)  (ACT sqrt + DVE reciprocal/mul)
  4. Main matmul, output-transposed orientation:
     outT[o, m] = sum_k W'^T[k, o] * xT[k, m], PSUM-accumulated over 32
     k-chunks; eviction fuses *scale_o + bias_o in one DVE tensor_scalar
     (per-partition scalars).
Host: layout transposes in numpy (no FLOPs), final gather + transpose.
"""

import numpy as np

import concourse.mybir as mybir
import concourse.tile as tile
from concourse import bacc
from concourse.bass_utils import run_bass_kernel_spmd
from concourse.masks import make_identity

OUT, IN, RANK = 4096, 4096, 16
SCALE = 2.0
NCORES = 8
OSH = OUT // NCORES          # 512 out features per core
P = 128
KO = IN // P                 # 32 k-chunks
KQ = 4                       # k-quarters for x streaming
KO_Q = KO // KQ              # 8 k-chunks per x tile
M = 4 * 2048                 # 8192 tokens
MCH = 512                    # tokens per x tile
NM = M // MCH                # 16 m-chunks
OC = OSH // P                # 4 o-chunks of 128
NRW = 256                    # ones rhs width for norm matmuls (f32r min)

F32 = mybir.dt.float32
F32R = mybir.dt.float32r
F16 = mybir.dt.float16


def _build():
    nc = bacc.Bacc(None, target_bir_lowering=False)
    xT = nc.dram_tensor("xT", [P, KO, M], F16, kind="ExternalInput")
    wT = nc.dram_tensor("wT", [P, KO, OSH], F32, kind="ExternalInput")
    aT = nc.dram_tensor("aT", [RANK, IN], F32R, kind="ExternalInput")
    bT = nc.dram_tensor("bT", [RANK, OSH], F32, kind="ExternalInput")
    wm = nc.dram_tensor("wm", [P, OC], F32, kind="ExternalInput")
    bc = nc.dram_tensor("bc", [P, OC], F32, kind="ExternalInput")
    outT = nc.dram_tensor("outT", [OSH, M], F32, kind="ExternalOutput")
    outT_v = outT.ap().rearrange("(oc p) m -> oc p m", p=P)

    with tile.TileContext(nc) as tc:
        with (
            tc.tile_pool(name="wr", bufs=1) as wrpool,
            tc.tile_pool(name="const", bufs=1) as cpool,
            tc.tile_pool(name="wv", bufs=2) as wvpool,
            tc.tile_pool(name="ach", bufs=4) as apool,
            tc.tile_pool(name="sq", bufs=2) as sqpool,
            tc.tile_pool(name="xs", bufs=4) as xpool,
            tc.tile_pool(name="os", bufs=3) as opool,
            tc.tile_pool(name="ps_mm", bufs=8, space="PSUM") as ps_mm,
        ):
            # ---- constants ----
            bt_f = cpool.tile([RANK, OSH], F32)
            nc.sync.dma_start(bt_f[:], bT.ap())
            bt2 = cpool.tile([RANK, OSH], F32R)
            nc.vector.tensor_scalar_mul(bt2[:], bt_f[:], SCALE)
            # all-ones lhsT for the norm matmuls. NOTE: f32r/f32 matmuls
            # whose PSUM output has fewer than 128 partitions compile but
            # produce a NEFF the runtime refuses to load - keep the lhsT
            # free dim (= output partitions) at 128.
            ones_f = cpool.tile([P, P], F32)
            nc.any.memset(ones_f[:], 1.0)
            ones_r = cpool.tile([P, P], F32R)
            nc.vector.tensor_copy(ones_r[:], ones_f[:])
            ident = cpool.tile([P, P], F32)
            make_identity(nc, ident)
            wm_col = cpool.tile([P, OC], F32)
            nc.sync.dma_start(wm_col[:], wm.ap())
            bias_col = cpool.tile([P, OC], F32)
            nc.sync.dma_start(bias_col[:], bc.ap())

            # ---- DoRA weight prep:
            # wr[:, ko] = W^T chunk + (2 B A)^T chunk   (PE + DVE add) ----
            wr = wrpool.tile([P, KO, OSH], F16)
            achs = []
            for kq in range(KQ):
                ach = apool.tile([RANK, KO_Q * P], F32R, name="ach")
                nc.sync.dma_start(
                    ach[:], aT.ap()[:, kq * KO_Q * P:(kq + 1) * KO_Q * P])
                achs.append(ach)
            # nr shares the "mm" tag (same bank size): the pool's 8 banks
            # cover prep (ba cycling + nr) and the main loop's 4 live
            # accumulation groups double-buffered across m-chunks
            nr = ps_mm.tile([P, OSH], F32, name="mm")
            # m-chunk 0's main matmuls are software-pipelined into the
            # prep loop: each kq's 32 main MMs need only the wr chunks
            # built in that kq and fill the PE gaps the DVE add/square
            # chain would otherwise leave
            pss0 = [ps_mm.tile([P, MCH], F32, name="mm") for _ in range(OC)]
            for kq in range(KQ):
                wv = wvpool.tile([P, KO_Q, OSH], F32)
                for k8 in range(KO_Q):
                    nc.sync.dma_start(
                        wv[:, k8], wT.ap()[:, kq * KO_Q + k8])
                for k8 in range(KO_Q):
                    ko = kq * KO_Q + k8
                    ba = ps_mm.tile([P, OSH], F32, name="mm")
                    nc.tensor.matmul(
                        ba[:], achs[kq][:, k8 * P:(k8 + 1) * P], bt2[:],
                        start=True, stop=True)
                    nc.vector.tensor_tensor(
                        wr[:, ko], wv[:, k8], ba[:], mybir.AluOpType.add)
                    # row-norm accumulation: norm2 row = ones^T @ wr^2,
                    # a single 32-matmul group in its own PSUM bank
                    # (concurrent groups must never share a bank -
                    # start=True clears whole-bank has_written state)
                    sq = sqpool.tile([P, OSH], F32R)
                    nc.vector.tensor_tensor(
                        sq[:], wr[:, ko], wr[:, ko], mybir.AluOpType.mult)
                    nc.tensor.matmul(
                        nr[:], ones_r[:], sq[:],
                        start=(ko == 0), stop=(ko == KO - 1))
                xt0 = xpool.tile([P, KO_Q, MCH], F16, name="xt")
                nc.sync.dma_start(
                    xt0[:], xT.ap()[:, kq * KO_Q:(kq + 1) * KO_Q, 0:MCH])
                for oc in range(OC):
                    for k8 in range(KO_Q):
                        nc.tensor.matmul(
                            pss0[oc][:],
                            wr[:, kq * KO_Q + k8, oc * P:(oc + 1) * P],
                            xt0[:, k8],
                            start=(kq == 0 and k8 == 0),
                            stop=(kq == KQ - 1 and k8 == KO_Q - 1))

            # ---- scale_col = wm / sqrt(norm2): every row of nr holds the
            # same 512 norms; PE-transpose 128-wide chunks of row space to
            # land them on o-partitions (no DRAM bounce - that reshape
            # path yields a NEFF the runtime refuses to load) ----
            sqc = cpool.tile([P, OC], F32)
            for oc in range(OC):
                nr_sb = sqpool.tile([P, P], F32, name="nrb")
                nc.vector.tensor_copy(nr_sb[:], nr[:, oc * P:(oc + 1) * P])
                pt = ps_mm.tile([P, P], F32, name="mm")
                nc.tensor.transpose(pt[:], nr_sb[:], ident[:])
                nc.scalar.activation(
                    sqc[:, oc:oc + 1], pt[:, 0:1],
                    mybir.ActivationFunctionType.Sqrt)
            rcp = cpool.tile([P, OC], F32)
            nc.vector.reciprocal(rcp[:], sqc[:])
            scale_col = cpool.tile([P, OC], F32)
            nc.vector.tensor_tensor(
                scale_col[:], wm_col[:], rcp[:], mybir.AluOpType.mult)

            # ---- m-chunk 0 eviction (matmuls ran inside the prep loop,
            # scale_col is now available) ----
            for oc in range(OC):
                ot0 = opool.tile([P, MCH], F32, name="ot")
                nc.vector.tensor_scalar(
                    ot0[:], pss0[oc][:],
                    scale_col[:, oc:oc + 1], bias_col[:, oc:oc + 1],
                    mybir.AluOpType.mult, mybir.AluOpType.add)
                nc.gpsimd.dma_start(outT_v[oc, :, 0:MCH], ot0[:])

            # ---- main matmul: outT[o, m] accumulated over k ----
            for mc in range(1, NM):
                pss = [ps_mm.tile([P, MCH], F32, name="mm")
                       for _ in range(OC)]
                for kq in range(KQ):
                    xt = xpool.tile([P, KO_Q, MCH], F16)
                    nc.sync.dma_start(
                        xt[:],
                        xT.ap()[:, kq * KO_Q:(kq + 1) * KO_Q,
                                mc * MCH:(mc + 1) * MCH])
                    for oc in range(OC):
                        for k8 in range(KO_Q):
                            nc.tensor.matmul(
                                pss[oc][:],
                                wr[:, kq * KO_Q + k8, oc * P:(oc + 1) * P],
                                xt[:, k8],
                                start=(kq == 0 and k8 == 0),
                                stop=(kq == KQ - 1 and k8 == KO_Q - 1))
                for oc in range(OC):
                    ot = opool.tile([P, MCH], F32)
                    nc.vector.tensor_scalar(
                        ot[:], pss[oc][:],
                        scale_col[:, oc:oc + 1], bias_col[:, oc:oc + 1],
                        mybir.AluOpType.mult, mybir.AluOpType.add)
                    # gpsimd (SWDGE) queue: keeps output writes off the
                    # sync queue that streams the x tiles
                    nc.gpsimd.dma_start(
                        outT_v[oc, :, mc * MCH:(mc + 1) * MCH], ot[:])
    nc.compile()
    return nc


def kernel(x, base_weight, base_bias, weight_m, dora_A, dora_B):
    x = np.asarray(x, dtype=np.float32)
    base_weight = np.asarray(base_weight, dtype=np.float32)
    base_bias = np.asarray(base_bias, dtype=np.float32)
    weight_m = np.asarray(weight_m, dtype=np.float32)
    dora_A = np.asarray(dora_A, dtype=np.float32)
    dora_B = np.asarray(dora_B, dtype=np.float32)

    B, S, _ = x.shape
    assert B * S == M and x.shape[2] == IN

    # xT[p, ko, m] = x[m, ko*128+p]  (fp16, shared across all cores)
    x2 = x.reshape(M, KO, P)
    xT = np.ascontiguousarray(x2.transpose(2, 1, 0)).astype(np.float16)

    in_maps = []
    for c in range(NCORES):
        sl = slice(c * OSH, (c + 1) * OSH)
        w_c = base_weight[sl]                                   # [OSH, IN]
        wT_c = np.ascontiguousarray(
            w_c.reshape(OSH, KO, P).transpose(2, 1, 0))         # [P, KO, OSH]
        bT_c = np.ascontiguousarray(dora_B[sl].T)               # [RANK, OSH]
        wm_c = np.ascontiguousarray(weight_m[sl].reshape(OC, P).T)
        bc_c = np.ascontiguousarray(base_bias[sl].reshape(OC, P).T)
        in_maps.append({
            "xT": xT,
            "wT": wT_c,
            "aT": dora_A,
            "bT": bT_c,
            "wm": wm_c,
            "bc": bc_c,
        })

    nc = _build()
    res = run_bass_kernel_spmd(nc, in_maps, core_ids=list(range(NCORES)))

    full = np.empty((OUT, M), dtype=np.float32)
    for c in range(NCORES):
        full[c * OSH:(c + 1) * OSH] = res.results[c]["outT"]
    return np.ascontiguousarray(full.T).reshape(B, S, OUT)


# revision 71
# speedup vs baseline: 1.1858x; 1.0067x over previous
"""DoRA Linear on 8 Trainium2 NeuronCores (Bass/Tile).

Reference computation (all fp32):
    new_v   = base_weight + SCALE * dora_B @ dora_A          [OUT, IN]
    scale_o = weight_m / ||new_v||_row                        [OUT]
    out     = x @ (scale_o[:, None] * new_v)^T + base_bias    [B, S, OUT]

Sharding: column-parallel over OUT across 8 cores (OUT/8 = 512 each).
base_weight, dora_B, weight_m, base_bias sharded; x, dora_A replicated.

Per-core device program (heavy math in float32r on the PE):
  1. Build W'^T = (W + SCALE*B@A)^T chunk-by-chunk in SBUF:
     PE matmul A^T@(2B^T) -> PSUM, DVE adds the W^T chunk and writes the
     f32r weight tile `wr` the main matmul consumes.
  2. Row norms: sq = wr*wr (DVE), then PE matmuls with sq as lhsT and an
     all-ones rhs (N=256) accumulate sum_i sq[i, o] over k-chunks,
     giving norms directly in o-partition column layout.  NOTE: f32/f32r
     matmuls whose PSUM output has fewer than 128 partitions compile but
     produce a NEFF the runtime refuses to load - keep M = 128.
  3. scale_col = weight_m / sqrt(norm2)  (ACT sqrt + DVE reciprocal/mul)
  4. Main matmul, output-transposed orientation:
     outT[o, m] = sum_k W'^T[k, o] * xT[k, m], PSUM-accumulated over 32
     k-chunks; eviction fuses *scale_o + bias_o in one DVE tensor_scalar
     (per-partition scalars).
Host: layout transposes in numpy (no FLOPs), final gather + transpose.
"""

import numpy as np

import concourse.mybir as mybir
import concourse.tile as tile
from concourse import bacc
from concourse.bass_utils import run_bass_kernel_spmd
from concourse.masks import make_identity

OUT, IN, RANK = 4096, 4096, 16
SCALE = 2.0
NCORES = 8
OSH = OUT // NCORES          # 512 out features per core
P = 128
KO = IN // P                 # 32 k-chunks
KQ = 4                       # k-quarters for x streaming
KO_Q = KO // KQ              # 8 k-chunks per x tile
M = 4 * 2048                 # 8192 tokens
MCH = 512                    # tokens per x tile
NM = M // MCH                # 16 m-chunks
OC = OSH // P                # 4 o-chunks of 128
NRW = 256                    # ones rhs width for norm matmuls (f32r min)

F32 = mybir.dt.float32
F32R = mybir.dt.float32r
F16 = mybir.dt.float16


def _build():
    nc = bacc.Bacc(None, target_bir_lowering=False)
    xT = nc.dram_tensor("xT", [P, KO, M], F16, kind="ExternalInput")
    wT = nc.dram_tensor("wT", [P, KO, OSH], F32, kind="ExternalInput")
    aT = nc.dram_tensor("aT", [RANK, IN], F16, kind="ExternalInput")
    bT = nc.dram_tensor("bT", [RANK, OSH], F32, kind="ExternalInput")
    wm = nc.dram_tensor("wm", [P, OC], F32, kind="ExternalInput")
    bc = nc.dram_tensor("bc", [P, OC], F32, kind="ExternalInput")
    outT = nc.dram_tensor("outT", [OSH, M], F32, kind="ExternalOutput")
    outT_v = outT.ap().rearrange("(oc p) m -> oc p m", p=P)

    with tile.TileContext(nc) as tc:
        with (
            tc.tile_pool(name="wr", bufs=1) as wrpool,
            tc.tile_pool(name="const", bufs=1) as cpool,
            tc.tile_pool(name="wv", bufs=2) as wvpool,
            tc.tile_pool(name="ach", bufs=4) as apool,
            tc.tile_pool(name="sq", bufs=2) as sqpool,
            tc.tile_pool(name="xs", bufs=6) as xpool,
            tc.tile_pool(name="os", bufs=3) as opool,
            tc.tile_pool(name="ps_mm", bufs=8, space="PSUM") as ps_mm,
        ):
            # ---- constants ----
            bt_f = cpool.tile([RANK, OSH], F32)
            nc.sync.dma_start(bt_f[:], bT.ap())
            bt2 = cpool.tile([RANK, OSH], F16)
            nc.vector.tensor_scalar_mul(bt2[:], bt_f[:], SCALE)
            # all-ones lhsT for the norm matmuls. NOTE: f32r/f32 matmuls
            # whose PSUM output has fewer than 128 partitions compile but
            # produce a NEFF the runtime refuses to load - keep the lhsT
            # free dim (= output partitions) at 128.
            ones_f = cpool.tile([P, P], F32)
            nc.any.memset(ones_f[:], 1.0)
            ones_r = cpool.tile([P, P], F16)
            nc.vector.tensor_copy(ones_r[:], ones_f[:])
            ident = cpool.tile([P, P], F32)
            make_identity(nc, ident)
            wm_col = cpool.tile([P, OC], F32)
            nc.sync.dma_start(wm_col[:], wm.ap())
            bias_col = cpool.tile([P, OC], F32)
            nc.sync.dma_start(bias_col[:], bc.ap())

            # ---- DoRA weight prep:
            # wr[:, ko] = W^T chunk + (2 B A)^T chunk   (PE + DVE add) ----
            wr = wrpool.tile([P, KO, OSH], F16)
            achs = []
            for kq in range(KQ):
                ach = apool.tile([RANK, KO_Q * P], F16, name="ach")
                nc.sync.dma_start(
                    ach[:], aT.ap()[:, kq * KO_Q * P:(kq + 1) * KO_Q * P])
                achs.append(ach)
            # nr shares the "mm" tag (same bank size): the pool's 8 banks
            # cover prep (ba cycling + nr) and the main loop's 4 live
            # accumulation groups double-buffered across m-chunks
            nr = ps_mm.tile([P, OSH], F32, name="mm")
            # m-chunk 0's main matmuls are software-pipelined into the
            # prep loop: each kq's 32 main MMs need only the wr chunks
            # built in that kq and fill the PE gaps the DVE add/square
            # chain would otherwise leave
            pss0 = [ps_mm.tile([P, MCH], F32, name="mm") for _ in range(OC)]
            for kq in range(KQ):
                wv = wvpool.tile([P, KO_Q, OSH], F32)
                for k8 in range(KO_Q):
                    nc.sync.dma_start(
                        wv[:, k8], wT.ap()[:, kq * KO_Q + k8])
                for k8 in range(KO_Q):
                    ko = kq * KO_Q + k8
                    ba = ps_mm.tile([P, OSH], F32, name="mm")
                    nc.tensor.matmul(
                        ba[:], achs[kq][:, k8 * P:(k8 + 1) * P], bt2[:],
                        start=True, stop=True)
                    nc.vector.tensor_tensor(
                        wr[:, ko], wv[:, k8], ba[:], mybir.AluOpType.add)
                    # row-norm accumulation: norm2 row = ones^T @ wr^2,
                    # a single 32-matmul group in its own PSUM bank
                    # (concurrent groups must never share a bank -
                    # start=True clears whole-bank has_written state)
                    sq = sqpool.tile([P, OSH], F16)
                    nc.vector.tensor_tensor(
                        sq[:], wr[:, ko], wr[:, ko], mybir.AluOpType.mult)
                    nc.tensor.matmul(
                        nr[:], ones_r[:], sq[:],
                        start=(ko == 0), stop=(ko == KO - 1))
                xt0 = xpool.tile([P, KO_Q, MCH], F16, name="xt")
                nc.sync.dma_start(
                    xt0[:], xT.ap()[:, kq * KO_Q:(kq + 1) * KO_Q, 0:MCH])
                for oc in range(OC):
                    for k8 in range(KO_Q):
                        nc.tensor.matmul(
                            pss0[oc][:],
                            wr[:, kq * KO_Q + k8, oc * P:(oc + 1) * P],
                            xt0[:, k8],
                            start=(kq == 0 and k8 == 0),
                            stop=(kq == KQ - 1 and k8 == KO_Q - 1))

            # ---- scale_col = wm / sqrt(norm2): every row of nr holds the
            # same 512 norms; PE-transpose 128-wide chunks of row space to
            # land them on o-partitions (no DRAM bounce - that reshape
            # path yields a NEFF the runtime refuses to load) ----
            sqc = cpool.tile([P, OC], F32)
            for oc in range(OC):
                nr_sb = sqpool.tile([P, P], F32, name="nrb")
                nc.vector.tensor_copy(nr_sb[:], nr[:, oc * P:(oc + 1) * P])
                pt = ps_mm.tile([P, P], F32, name="mm")
                nc.tensor.transpose(pt[:], nr_sb[:], ident[:])
                nc.scalar.activation(
                    sqc[:, oc:oc + 1], pt[:, 0:1],
                    mybir.ActivationFunctionType.Sqrt)
            rcp = cpool.tile([P, OC], F32)
            nc.vector.reciprocal(rcp[:], sqc[:])
            scale_col = cpool.tile([P, OC], F32)
            nc.vector.tensor_tensor(
                scale_col[:], wm_col[:], rcp[:], mybir.AluOpType.mult)

            # ---- m-chunk 0 eviction (matmuls ran inside the prep loop,
            # scale_col is now available) ----
            for oc in range(OC):
                ot0 = opool.tile([P, MCH], F32, name="ot")
                nc.vector.tensor_scalar(
                    ot0[:], pss0[oc][:],
                    scale_col[:, oc:oc + 1], bias_col[:, oc:oc + 1],
                    mybir.AluOpType.mult, mybir.AluOpType.add)
                nc.gpsimd.dma_start(outT_v[oc, :, 0:MCH], ot0[:])

            # ---- main matmul: outT[o, m] accumulated over k ----
            for mc in range(1, NM):
                pss = [ps_mm.tile([P, MCH], F32, name="mm")
                       for _ in range(OC)]
                for kq in range(KQ):
                    xt = xpool.tile([P, KO_Q, MCH], F16)
                    nc.sync.dma_start(
                        xt[:],
                        xT.ap()[:, kq * KO_Q:(kq + 1) * KO_Q,
                                mc * MCH:(mc + 1) * MCH])
                    for oc in range(OC):
                        for k8 in range(KO_Q):
                            nc.tensor.matmul(
                                pss[oc][:],
                                wr[:, kq * KO_Q + k8, oc * P:(oc + 1) * P],
                                xt[:, k8],
                                start=(kq == 0 and k8 == 0),
                                stop=(kq == KQ - 1 and k8 == KO_Q - 1))
                for oc in range(OC):
                    ot = opool.tile([P, MCH], F32)
                    nc.vector.tensor_scalar(
                        ot[:], pss[oc][:],
                        scale_col[:, oc:oc + 1], bias_col[:, oc:oc + 1],
                        mybir.AluOpType.mult, mybir.AluOpType.add)
                    # gpsimd (SWDGE) queue: keeps output writes off the
                    # sync queue that streams the x tiles
                    nc.gpsimd.dma_start(
                        outT_v[oc, :, mc * MCH:(mc + 1) * MCH], ot[:])
    nc.compile()
    return nc


def kernel(x, base_weight, base_bias, weight_m, dora_A, dora_B):
    x = np.asarray(x, dtype=np.float32)
    base_weight = np.asarray(base_weight, dtype=np.float32)
    base_bias = np.asarray(base_bias, dtype=np.float32)
    weight_m = np.asarray(weight_m, dtype=np.float32)
    dora_A = np.asarray(dora_A, dtype=np.float32)
    dora_B = np.asarray(dora_B, dtype=np.float32)

    B, S, _ = x.shape
    assert B * S == M and x.shape[2] == IN

    # xT[p, ko, m] = x[m, ko*128+p]  (fp16, shared across all cores)
    x2 = x.reshape(M, KO, P)
    xT = np.ascontiguousarray(x2.transpose(2, 1, 0)).astype(np.float16)

    in_maps = []
    for c in range(NCORES):
        sl = slice(c * OSH, (c + 1) * OSH)
        w_c = base_weight[sl]                                   # [OSH, IN]
        wT_c = np.ascontiguousarray(
            w_c.reshape(OSH, KO, P).transpose(2, 1, 0))         # [P, KO, OSH]
        bT_c = np.ascontiguousarray(dora_B[sl].T)               # [RANK, OSH]
        wm_c = np.ascontiguousarray(weight_m[sl].reshape(OC, P).T)
        bc_c = np.ascontiguousarray(base_bias[sl].reshape(OC, P).T)
        in_maps.append({
            "xT": xT,
            "wT": wT_c,
            "aT": dora_A.astype(np.float16),
            "bT": bT_c,
            "wm": wm_c,
            "bc": bc_c,
        })

    nc = _build()
    res = run_bass_kernel_spmd(nc, in_maps, core_ids=list(range(NCORES)))

    full = np.empty((OUT, M), dtype=np.float32)
    for c in range(NCORES):
        full[c * OSH:(c + 1) * OSH] = res.results[c]["outT"]
    return np.ascontiguousarray(full.T).reshape(B, S, OUT)
